# revision 23
# baseline (speedup 1.0000x reference)
"""MultiHeadAttention (8 heads, d_emb=512, d_hid=64, seq 2048, batch 8) on 8
Trainium2 NeuronCores.

Sharding: data parallel over batch - core i computes batch element i fully
(weights replicated, no collectives).

Per-core pipeline, ACT-roofline design (33.5M softmax exps, ~266us):
  layout:  x^T and Q/K/V weights host-prescaled x16 and cast fp8e4 (errors
           land on the attention branch, diluted ~100x by the residual).
           exp scale folds the 1/256 back out.
  proj:    Q/K/V projections as fp8 DoubleRow matmuls (2 e-chunks per
           instruction, 0.5 cyc/row). Q/K bias fused in the PSUM->SBUF
           eviction (bf16 out); V bias -> fp8 V8.
  attn:    16 blocks = (s-half, pair, head-half); per t: one bf16 scores
           matmul pair into a [128,1024] PSUM tile, one exp -> fp8 SBUF
           (paired per two key-tiles); ctx = fp8 DoubleRow over t-pairs;
           softmax denominators via an all-ones fp8 DoubleRow matmul into a
           64-row PSUM tile (every row = den), so normalization is just
           reciprocal + elementwise multiply - no partition broadcast.
  norm:    deferred into the next block's early slots (no boundary stall);
           CCT is [64, head, S] (stage-3 contracts 64-row head chunks), so
           no partition shifts anywhere.
  out:     out = sum_h CCT[h].T @ Wo[h] (+bo rank-1), residual, LayerNorm
           via bn_stats/bn_aggr, DMA out. Weights/X/out DMAs spread over
           the four engine DGE queues, ordered so the first scores land
           ~12us in.
"""

import copy
import json
import sys
import types

import numpy as np

for _p in ("/opt/trn_rl_repo", "/root/.axon_site/_ro/trn_rl_repo"):
    if _p not in sys.path:
        sys.path.append(_p)

import concourse.bass as bass
import concourse.mybir as mybir
import concourse.tile as tile

P = 128
S = 2048  # sequence length
E = 512  # embedding dim
H = 8  # heads
D = 64  # head dim
NP = H // 2  # head pairs
ST = S // P  # seq tiles
ET = E // P  # embedding tiles
WS = 16.0  # host weight prescale (Q/K/V)
SCALE_EXP = (1.0 / 8.0) / (WS * WS)  # 1/sqrt(D) folded with the prescales
LN_EPS = 1e-5
F32 = mybir.dt.float32
BF16 = mybir.dt.bfloat16
FP8 = mybir.dt.float8e4
FP8E5 = mybir.dt.float8e5
AF = mybir.ActivationFunctionType
OP = mybir.AluOpType
PM = mybir.MatmulPerfMode


# --------------------------------------------------------------------------
# walrus in this build accepts only ONE sync-wait per instruction; Tile's sem
# assignment can attach several (e.g. the kernel-tail drain). Splitting the
# extra waits onto preceding NoOps on the same engine is semantically
# identical (engine streams execute in order).
def _split_waits(m, max_waits=1):
    for fn in m.get("functions", []):
        for blk in fn.get("blocks", []):
            new_insts = []
            for inst in blk.get("instructions", []):
                sync = inst.get("sync_info") or {}
                ow = sync.get("on_wait") or []
                if len(ow) > max_waits:
                    extra = ow[:-max_waits]
                    inst["sync_info"]["on_wait"] = ow[-max_waits:]
                    for ci in range(0, len(extra), max_waits):
                        nop = copy.deepcopy(inst)
                        nop["name"] = f"{inst['name']}ws{ci}"
                        nop["opcode"] = "NoOp"
                        nop["ins"] = []
                        nop["outs"] = []
                        nop["is_reset_sema"] = False
                        nop["sync_info"] = {
                            "on_update": [],
                            "on_wait": extra[ci : ci + max_waits],
                        }
                        new_insts.append(nop)
                new_insts.append(inst)
            blk["instructions"] = new_insts
    return m


def _patch_to_json(nc):
    orig = nc.to_json_bytes

    def patched(self):
        return json.dumps(_split_waits(json.loads(orig()))).encode()

    nc.to_json_bytes = types.MethodType(patched, nc)


def _bcast_ap(ap, parts):
    """[N]-shaped DRAM AP -> [parts, N] via zero-stride partition dim."""
    return bass.AP(
        tensor=ap.tensor, offset=ap.offset, ap=[[0, parts]] + list(ap.ap[-1:])
    )


# --------------------------------------------------------------------------
def build_nc():
    nc = bass.Bass()
    xD = nc.declare_dram_parameter("x", [S, E], F32, isOutput=False)
    # smA: bqk [P,8] ++ bv broadcast [P,512]; smB: gamma/beta/bo broadcasts
    smAD = nc.declare_dram_parameter("smA", [P, 8 + E], F32, isOutput=False)
    smBD = nc.declare_dram_parameter("smB", [P, 3 * E], F32, isOutput=False)
    # host-preprocessed layouts: x^T fp8, e-major prescaled fp8 weights,
    # Wo in 64-row head chunks (bf16, already /WS)
    xTD = nc.declare_dram_parameter("xT", [E, S], FP8, isOutput=False)
    wqpD = nc.declare_dram_parameter("Wq_p", [E, H * D], FP8, isOutput=False)
    wkpD = nc.declare_dram_parameter("Wk_p", [E, H * D], FP8, isOutput=False)
    wvpD = nc.declare_dram_parameter("Wv_p", [E, H * D], FP8, isOutput=False)
    wopD = nc.declare_dram_parameter("Wo_p", [D, H, E], FP8E5, isOutput=False)
    outD = nc.declare_dram_parameter("out", [S, E], F32, isOutput=True)

    qs = [nc.sync, nc.scalar, nc.gpsimd]  # DGE queues

    with tile.TileContext(nc) as tc:
        with (
            tc.tile_pool(name="persist", bufs=1) as persist,
        ):
            X = persist.tile([P, ST, E], F32, name="Xsb")
            XT = persist.tile([P, ET, S], FP8, name="XTsb")
            Wq_sb = persist.tile([P, ET, H, D], FP8, name="Wq_sb")
            Wk_sb = persist.tile([P, ET, H, D], FP8, name="Wk_sb")
            Wv_sb = persist.tile([P, ET, H * D], FP8, name="Wv_sb")
            Wo_sb = persist.tile([D, H, E], FP8E5, name="Wo_sb")
            smA = persist.tile([P, 8 + E], F32, name="smA")
            smB = persist.tile([P, 3 * E], F32, name="smB")
            bo_row = persist.tile([1, E], BF16, name="bo_row")
            ones_bf = persist.tile([1, E], BF16, name="ones_bf")
            ones8 = persist.tile([P, 2, D], FP8, name="ones8")
            bqk = smA[:, 0:8].rearrange("p (a b) -> p a b", a=2)
            bv_bc = smA[:, 8 : 8 + E].rearrange("p (h d) -> p h d", h=H)
            gamma_bc = smB[:, 0:E]
            beta_bc = smB[:, E : 2 * E]
            QT = persist.tile([P, NP, S], BF16, name="QTsb")
            KT = persist.tile([P, NP, S], BF16, name="KTsb")
            V8 = persist.tile([P, ST, H, D], FP8, name="V8sb")
            CCT = persist.tile([D, H, S], FP8, name="CCTsb")

            # ---------------- stage 0: loads + pair-0 Q/K -------------------
            with (
                tc.tile_pool(name="qkp", bufs=4, space="PSUM") as qkp,
            ):
                nc.vector.memset(ones8, 1.0)
                nc.vector.memset(ones_bf, 1.0)

                # PE warmup while the first DMAs land: HAM un-throttles after
                # ~3.4us of sustained activity; sized to end near the first
                # weight/x^T arrivals so the first projection isn't queued
                # behind idle warmups
                for _w in range(6):
                    warm = qkp.tile([P, 512], F32, tag="SC", name="warm")
                    nc.tensor.matmul(
                        warm, lhsT=ones_bf[:, 0:P], rhs=ones_bf[:, 0:512],
                        start=True, stop=True,
                    )

                # wave 1: one big DMA per queue slot (HWDGE charges ~0.6us
                # fixed per DMACopy; transfers fan out over 16 engines, so
                # few-and-large wins). sync: Wq + x^T half 1; scalar: Wk +
                # x^T half 2; gpsimd: biases + Wv.
                xt_re = lambda lo, hi: xTD[:, lo:hi].rearrange(
                    "(et p) s -> p et s", p=P
                )
                # pair-0 weight columns first: the shared DMA fabric drains
                # transfers in issue order, so the first-scores set must lead
                w_re = lambda wD, lo, hi: wD[:, lo:hi].rearrange(
                    "(et p) hd -> p et hd", p=P
                ).rearrange("p et (h d) -> p et h d", h=2 * (hi - lo) // P)
                nc.sync.dma_start(out=XT[:, :, 0:1024], in_=xt_re(0, 1024))
                nc.scalar.dma_start(
                    out=Wk_sb[:, :, 0:2], in_=w_re(wkpD, 0, P)
                )
                nc.gpsimd.dma_start(out=smA, in_=smAD[:])
                nc.sync.dma_start(
                    out=Wq_sb[:, :, 0:2], in_=w_re(wqpD, 0, P)
                )
                nc.scalar.dma_start(
                    out=Wk_sb[:, :, 2:8], in_=w_re(wkpD, P, 4 * P)
                )
                nc.gpsimd.dma_start(
                    out=Wq_sb[:, :, 2:8], in_=w_re(wqpD, P, 4 * P)
                )
                nc.scalar.dma_start(
                    out=XT[:, :, 1024:2048], in_=xt_re(1024, 2048)
                )
                nc.scalar.dma_start(
                    out=Wv_sb,
                    in_=wvpD[:].rearrange("(et p) hd -> p et hd", p=P),
                )

                # pair-0 projections, first-scores chunks only: Q cc0, K cc0,
                # Q cc1 (K cc1-3 stream in as block-0 chunks; Q cc2-3 later)
                for qk, cc in ((0, 0), (1, 0), (0, 1)):
                    wsb = Wq_sb if qk == 0 else Wk_sb
                    qt = QT if qk == 0 else KT
                    pq = qkp.tile([P, 512], F32, tag="SC", name="pq")
                    for ep in range(2):
                        nc.tensor.matmul(
                            pq,
                            lhsT=wsb[:, 2 * ep : 2 * ep + 2, 0:2]
                            .rearrange("p e h d -> p e (h d)"),
                            rhs=XT[
                                :, 2 * ep : 2 * ep + 2,
                                cc * 512 : (cc + 1) * 512,
                            ],
                            start=(ep == 0),
                            stop=(ep == 1),
                            perf_mode=PM.DoubleRow,
                        )
                    nc.vector.tensor_scalar_add(
                        qt[:, 0, cc * 512 : (cc + 1) * 512],
                        pq,
                        bqk[:, qk, 0:1],
                    )

            # ---------------- attention: 16 blocks of (sh, pp, hl) ----------
            with (
                tc.tile_pool(name="expp", bufs=5) as expp,
                tc.tile_pool(name="scp", bufs=2, space="PSUM") as scp,
                tc.tile_pool(name="ctxp", bufs=1, space="PSUM") as ctxp,
                tc.tile_pool(name="denp", bufs=1, space="PSUM") as denp,
                tc.tile_pool(name="recp", bufs=3) as recp,
                tc.tile_pool(name="outp", bufs=8) as outp,
                tc.tile_pool(name="statp", bufs=8) as statp,
            ):
                eps_t = statp.tile([P, 1], F32, tag="eps", bufs=1)
                nc.vector.memset(eps_t, LN_EPS)

                # ---- chunk emitters (scheduled through PE idle slots) ----
                def qk_chunk(pp, qk, cc):
                    def emit():
                        wsb = Wq_sb if qk == 0 else Wk_sb
                        qt = QT if qk == 0 else KT
                        pq = scp.tile([P, 1024], F32, tag="SC", name="pq2")
                        for ep in range(2):
                            nc.tensor.matmul(
                                pq[:, 0:512],
                                lhsT=wsb[:, 2 * ep : 2 * ep + 2,
                                         2 * pp : 2 * pp + 2]
                                .rearrange("p e h d -> p e (h d)"),
                                rhs=XT[
                                    :, 2 * ep : 2 * ep + 2,
                                    cc * 512 : (cc + 1) * 512,
                                ],
                                start=(ep == 0),
                                stop=(ep == 1),
                                perf_mode=PM.DoubleRow,
                            )
                        nc.vector.tensor_scalar_add(
                            qt[:, pp, cc * 512 : (cc + 1) * 512],
                            pq[:, 0:512],
                            bqk[:, qk, pp : pp + 1],
                        )

                    return emit

                def v_chunk(st):
                    def emit():
                        pv = scp.tile([P, 1024], F32, tag="SC", name="pv")
                        for ep in range(2):
                            nc.tensor.matmul(
                                pv[:, 0:512],
                                lhsT=XT[:, 2 * ep : 2 * ep + 2,
                                        st * P : (st + 1) * P],
                                rhs=Wv_sb[:, 2 * ep : 2 * ep + 2, :],
                                start=(ep == 0),
                                stop=(ep == 1),
                                perf_mode=PM.DoubleRow,
                            )
                        nc.vector.tensor_tensor(
                            V8[:, st],
                            pv[:, 0:512].rearrange("p (h d) -> p h d", h=H),
                            bv_bc,
                            OP.add,
                        )

                    return emit

                def st3_mm(st):
                    po = scp.tile([P, 1024], F32, tag="SC", name="po3")
                    for j in range(H // 2):
                        nc.tensor.matmul(
                            po[:, 0:E],
                            lhsT=CCT[:, 2 * j : 2 * j + 2,
                                     st * P : (st + 1) * P],
                            rhs=Wo_sb[:, 2 * j : 2 * j + 2, :],
                            start=(j == 0),
                            stop=False,
                            perf_mode=PM.DoubleRow,
                        )
                    nc.tensor.matmul(
                        po[:, 0:E], lhsT=ones_bf[:, 0:P], rhs=bo_row,
                        start=False, stop=True,
                    )
                    y = outp.tile([P, E], F32, tag="y", name="y")
                    nc.vector.tensor_add(y, po[:, 0:E], X[:, st])
                    return y

                def st3_ln_out(st, y, mu, rstd, tail=False):
                    nc.vector.tensor_scalar(
                        y, y, mu, rstd, OP.subtract, OP.mult
                    )
                    nc.vector.tensor_tensor(y, y, gamma_bc, OP.mult)
                    nc.gpsimd.tensor_tensor(y, y, beta_bc, OP.add)
                    nc.sync.dma_start(
                        out=outD[st * P : (st + 1) * P, :], in_=y
                    )

                def st3_chunk(st):
                    def emit():
                        y = st3_mm(st)
                        stats = statp.tile([P, 6], F32, tag="st", name="stats")
                        nc.vector.bn_stats(out=stats, in_=y)
                        mv = statp.tile([P, 2], F32, tag="mv", name="mv")
                        nc.vector.bn_aggr(out=mv, in_=stats)
                        rstd = statp.tile([P, 1], F32, tag="rs", name="rstd")
                        # rstd = exp(-0.5*ln(var+eps)): Ln/Exp share one ACT
                        # table set, so no table reloads between softmax exps
                        nc.scalar.activation(
                            out=rstd, in_=mv[:, 1:2], func=AF.Ln, bias=eps_t
                        )
                        nc.scalar.activation(
                            out=rstd, in_=rstd, func=AF.Exp, scale=-0.5
                        )
                        st3_ln_out(st, y, mv[:, 0:1], rstd)

                    return emit

                def st3_tail(st):
                    # epilogue variant: mean/var via ACT accumulators (ACT is
                    # idle after the last exp; DVE is the tail critical path)
                    y = st3_mm(st)
                    dump = statp.tile([P, E], BF16, tag="dmp", name="dump")
                    mu = statp.tile([P, 1], F32, tag="mu", name="mu")
                    nc.scalar.activation(
                        out=dump, in_=y, func=AF.Copy, scale=1.0 / E,
                        accum_out=mu,
                    )
                    dump2 = statp.tile([P, E], F32, tag="dm2", name="dump2")
                    a2 = statp.tile([P, 1], F32, tag="a2", name="a2")
                    nc.scalar.activation(
                        out=dump2, in_=y, func=AF.Square, scale=1.0 / E,
                        accum_out=a2,
                    )
                    musq = statp.tile([P, 1], F32, tag="mq", name="musq")
                    nc.vector.tensor_tensor(musq, mu, mu, OP.mult)
                    var = statp.tile([P, 1], F32, tag="va", name="var")
                    # var = E*sum((y/E)^2) - mu^2
                    nc.vector.tensor_scalar(
                        var, a2, float(E), musq, OP.mult, OP.subtract
                    )
                    rstd = statp.tile([P, 1], F32, tag="rs", name="rstd")
                    nc.scalar.activation(
                        out=rstd, in_=var, func=AF.Ln, bias=eps_t
                    )
                    nc.scalar.activation(
                        out=rstd, in_=rstd, func=AF.Exp, scale=-0.5
                    )
                    st3_ln_out(st, y, mu, rstd, tail=True)

                def wo_load():
                    def emit():
                        nc.gpsimd.dma_start(out=Wo_sb, in_=wopD[:])
                        nc.sync.dma_start(out=smB, in_=smBD[:])
                        nc.gpsimd.tensor_copy(
                            out=bo_row, in_=smB[0:1, 2 * E : 3 * E]
                        )

                    return emit

                def x_load(q):
                    def emit():
                        xDr = xD[:].rearrange("(st p) e -> p st e", p=P)
                        (nc.sync if q % 2 else nc.gpsimd).dma_start(
                            out=X[:, 4 * q : 4 * q + 4],
                            in_=xDr[:, 4 * q : 4 * q + 4],
                        )

                    return emit

                # ---- static chunk schedule ----
                pre = {}

                def add(b, t, fn):
                    pre.setdefault((b, t), []).append(fn)

                # pair-0 K tail: keys 512+ first consumed at t4/t8/t12
                add(0, 2, qk_chunk(0, 1, 1))
                add(0, 6, qk_chunk(0, 1, 2))
                add(0, 10, qk_chunk(0, 1, 3))
                for st in range(13):  # V projections: block 0
                    add(0, 3 + st, v_chunk(st))
                add(1, 1, v_chunk(13))
                add(1, 2, v_chunk(14))
                add(1, 3, v_chunk(15))
                qkseq = {  # pairs 1-3: K cc0-3 then Q cc0-1
                    1: [(0, 11), (0, 12), (0, 13), (0, 14), (1, 4), (1, 8)],
                    2: [(2, 4), (2, 8), (2, 12), (3, 4), (3, 8), (3, 12)],
                    3: [(4, 4), (4, 8), (4, 12), (5, 4), (5, 8), (5, 12)],
                }
                for pp, slots in qkseq.items():
                    needs = [(pp, 1, 0), (pp, 1, 1), (pp, 1, 2), (pp, 1, 3),
                             (pp, 0, 0), (pp, 0, 1)]
                    for (b, t), (p_, qk, cc) in zip(slots, needs):
                        add(b, t, qk_chunk(p_, qk, cc))
                qt23 = {0: [(5, 10), (6, 4)], 1: [(6, 8), (6, 12)],
                        2: [(8, 4), (8, 8)], 3: [(10, 4), (10, 8)]}
                for pp, slots in qt23.items():
                    for (b, t), cc in zip(slots, (2, 3)):
                        add(b, t, qk_chunk(pp, 0, cc))
                add(1, 6, wo_load())
                for q in range(4):
                    add(2 + q, 6, x_load(q))
                for st in range(8):  # stage-3 for the first s-half
                    add(8 + st, 8, st3_chunk(st))

                # ---- the 16 attention blocks ----
                blocks = [
                    (sh, pp, hl)
                    for sh in range(2)
                    for pp in range(NP)
                    for hl in range(2)
                ]
                prev = None  # (cx, den_t, h, s0) awaiting normalize

                def emit_ctx_den(cx, den_t, et2, k, h):
                    for cc in range(2):
                        nc.tensor.matmul(
                            cx[:, cc * 512 : (cc + 1) * 512],
                            lhsT=V8[:, 2 * k : 2 * k + 2, h, :],
                            rhs=et2[:, :, cc * 512 : (cc + 1) * 512],
                            start=(k == 0),
                            stop=(k == 7),
                            perf_mode=PM.DoubleRow,
                        )
                        nc.tensor.matmul(
                            den_t[:, cc * 512 : (cc + 1) * 512],
                            lhsT=ones8,
                            rhs=et2[:, :, cc * 512 : (cc + 1) * 512],
                            start=(k == 0),
                            stop=(k == 7),
                            perf_mode=PM.DoubleRow,
                        )

                def normalize_half(state, half):
                    cx_p, den_p, h_p, s0_p = state
                    lo, hi = half * 512, (half + 1) * 512
                    rec = recp.tile([D, 512], F32, tag="rec", name="rec")
                    nc.vector.reciprocal(rec, den_p[:, lo:hi])
                    nc.vector.tensor_tensor(
                        CCT[:, h_p, s0_p + lo : s0_p + hi],
                        cx_p[:, lo:hi],
                        rec,
                        OP.mult,
                    )

                def normalize(state):
                    cx_p, den_p, h_p, s0_p = state
                    rec = recp.tile([D, 1024], F32, tag="recf", name="recf")
                    nc.vector.reciprocal(rec, den_p)
                    nc.vector.tensor_tensor(
                        CCT[:, h_p, s0_p : s0_p + 1024], cx_p, rec, OP.mult
                    )

                for b, (sh, pp, hl) in enumerate(blocks):
                    s0 = sh * 1024
                    h = 2 * pp + hl
                    lo, hi = D * hl, D * (hl + 1)
                    cx = ctxp.tile([D, 1024], F32, tag="ctx", name="cx")
                    den_t = denp.tile([D, 1024], F32, tag="den", name="den")
                    ets = {}
                    for t in range(ST):
                        site = pre.get((b, t), [])
                        for fn in site:
                            fn()
                        if len(site) % 2:
                            # keep the SC ring parity: scores(t) must not
                            # land in scores(t-1)'s slot
                            scp.tile([P, 1024], F32, tag="SC", name="par")
                        if t == 1 and prev is not None:
                            normalize(prev)
                            prev = None
                        sc = scp.tile([P, 1024], F32, tag="SC", name="sc")
                        for cc in range(2):
                            nc.tensor.matmul(
                                sc[:, cc * 512 : (cc + 1) * 512],
                                lhsT=KT[lo:hi, pp, t * P : (t + 1) * P],
                                rhs=QT[
                                    lo:hi, pp,
                                    s0 + cc * 512 : s0 + (cc + 1) * 512,
                                ],
                                start=True,
                                stop=True,
                            )
                        if t % 2 == 0:
                            ets[t // 2] = expp.tile(
                                [P, 2, 1024], FP8, tag="expT", name="et"
                            )
                        nc.scalar.activation(
                            out=ets[t // 2][:, t % 2],
                            in_=sc,
                            func=AF.Exp,
                            scale=SCALE_EXP,
                        )
                        if t == 0 and b > 0:
                            # prev block's last ctx/den pair: emitted after
                            # this block's first scores so PE never stalls
                            # the exp pipeline on exp(t15) of the old block
                            pcx, pden, ph, ps0, pets = pending
                            emit_ctx_den(pcx, pden, pets, 7, ph)
                        if t >= 4 and t % 2 == 0:
                            k = (t - 4) // 2
                            emit_ctx_den(cx, den_t, ets.pop(k), k, h)
                    emit_ctx_den(cx, den_t, ets.pop(6), 6, h)
                    pending = (cx, den_t, h, s0, ets.pop(7))
                    prev = (cx, den_t, h, s0)

                # drain: last block's pair 7, its normalize, tail stage-3
                pcx, pden, ph, ps0, pets = pending
                emit_ctx_den(pcx, pden, pets, 7, ph)
                normalize_half(prev, 0)
                normalize_half(prev, 1)
                tc.cur_priority += 20000
                for st in range(8, ST):
                    st3_tail(st)
                tc.cur_priority -= 20000

    _patch_to_json(nc)
    return nc


_NC_CACHE = None


def _get_nc():
    global _NC_CACHE
    if _NC_CACHE is None:
        _NC_CACHE = build_nc()
    return _NC_CACHE


def kernel(**inputs) -> np.ndarray:
    import ml_dtypes
    from concourse.bass_utils import run_bass_kernel_spmd

    BF = ml_dtypes.bfloat16
    E4 = ml_dtypes.float8_e4m3fn
    nc = _get_nc()
    x = np.asarray(inputs["x"], dtype=np.float32)
    B = x.shape[0]

    def f32(k, scale=1.0):
        return np.ascontiguousarray(
            np.asarray(inputs[k], dtype=np.float32) * scale
        )

    def perm_w8(k):  # [H, E, D] -> [E, H*D] fp8, prescaled
        w = np.asarray(inputs[k], dtype=np.float32) * WS
        return np.ascontiguousarray(
            w.transpose(1, 0, 2).reshape(E, H * D).astype(E4)
        )

    bqk = np.stack(
        [
            np.asarray(inputs["bq"], np.float32).reshape(NP, P).T * WS,
            np.asarray(inputs["bk"], np.float32).reshape(NP, P).T * WS,
        ],
        axis=1,
    ).reshape(P, 8)
    bv_bc = np.broadcast_to(
        np.asarray(inputs["bv"], np.float32).reshape(1, H * D) * WS, (P, H * D)
    )
    smA = np.ascontiguousarray(np.concatenate([bqk, bv_bc], axis=1))
    smB = np.ascontiguousarray(
        np.stack(
            [
                np.broadcast_to(f32(k).reshape(1, E), (P, E))
                for k in ("gamma", "beta", "bo")
            ],
            axis=1,
        ).reshape(P, 3 * E)
    )
    wo = np.asarray(inputs["Wo"], np.float32) / WS  # [H*D, E]
    wo = wo.reshape(H, D, E).transpose(1, 0, 2)  # [D, H, E]
    shared = {
        "Wq_p": perm_w8("Wq"),
        "Wk_p": perm_w8("Wk"),
        "Wv_p": perm_w8("Wv"),
        "Wo_p": np.ascontiguousarray(wo.astype(ml_dtypes.float8_e5m2)),
        "smA": smA,
        "smB": smB,
    }
    in_maps = []
    for b in range(B):
        xb = np.ascontiguousarray(x[b])
        in_maps.append(
            {
                "x": xb,
                "xT": np.ascontiguousarray(xb.T.astype(E4)),
                **shared,
            }
        )
    res = run_bass_kernel_spmd(nc, in_maps, core_ids=list(range(B)))
    return np.stack([res.results[b]["out"] for b in range(B)], axis=0)


# revision 24
# speedup vs baseline: 1.0252x; 1.0252x over previous
"""MultiHeadAttention (8 heads, d_emb=512, d_hid=64, seq 2048, batch 8) on 8
Trainium2 NeuronCores.

Sharding: data parallel over batch - core i computes batch element i fully
(weights replicated, no collectives).

Per-core pipeline, ACT-roofline design (33.5M softmax exps, ~266us):
  layout:  x^T and Q/K/V weights host-prescaled x16 and cast fp8e4 (errors
           land on the attention branch, diluted ~100x by the residual).
           exp scale folds the 1/256 back out.
  proj:    Q/K/V projections as fp8 DoubleRow matmuls (2 e-chunks per
           instruction, 0.5 cyc/row). Q/K bias fused in the PSUM->SBUF
           eviction (bf16 out); V bias -> fp8 V8.
  attn:    16 blocks = (s-half, pair, head-half); per t: one bf16 scores
           matmul pair into a [128,1024] PSUM tile, one exp -> fp8 SBUF
           (paired per two key-tiles); ctx = fp8 DoubleRow over t-pairs;
           softmax denominators via an all-ones fp8 DoubleRow matmul into a
           64-row PSUM tile (every row = den), so normalization is just
           reciprocal + elementwise multiply - no partition broadcast.
  norm:    deferred into the next block's early slots (no boundary stall);
           CCT is [64, head, S] (stage-3 contracts 64-row head chunks), so
           no partition shifts anywhere.
  out:     out = sum_h CCT[h].T @ Wo[h] (+bo rank-1), residual, LayerNorm
           via bn_stats/bn_aggr, DMA out. Weights/X/out DMAs spread over
           the four engine DGE queues, ordered so the first scores land
           ~12us in.
"""

import copy
import json
import sys
import types

import numpy as np

for _p in ("/opt/trn_rl_repo", "/root/.axon_site/_ro/trn_rl_repo"):
    if _p not in sys.path:
        sys.path.append(_p)

import concourse.bass as bass
import concourse.mybir as mybir
import concourse.tile as tile

P = 128
S = 2048  # sequence length
E = 512  # embedding dim
H = 8  # heads
D = 64  # head dim
NP = H // 2  # head pairs
ST = S // P  # seq tiles
ET = E // P  # embedding tiles
WS = 16.0  # host weight prescale (Q/K/V)
SCALE_EXP = (1.0 / 8.0) / (WS * WS)  # 1/sqrt(D) folded with the prescales
LN_EPS = 1e-5
F32 = mybir.dt.float32
BF16 = mybir.dt.bfloat16
FP8 = mybir.dt.float8e4
FP8E5 = mybir.dt.float8e5
AF = mybir.ActivationFunctionType
OP = mybir.AluOpType
PM = mybir.MatmulPerfMode


# --------------------------------------------------------------------------
# walrus in this build accepts only ONE sync-wait per instruction; Tile's sem
# assignment can attach several (e.g. the kernel-tail drain). Splitting the
# extra waits onto preceding NoOps on the same engine is semantically
# identical (engine streams execute in order).
def _split_waits(m, max_waits=1):
    for fn in m.get("functions", []):
        for blk in fn.get("blocks", []):
            new_insts = []
            for inst in blk.get("instructions", []):
                sync = inst.get("sync_info") or {}
                ow = sync.get("on_wait") or []
                if len(ow) > max_waits:
                    extra = ow[:-max_waits]
                    inst["sync_info"]["on_wait"] = ow[-max_waits:]
                    for ci in range(0, len(extra), max_waits):
                        nop = copy.deepcopy(inst)
                        nop["name"] = f"{inst['name']}ws{ci}"
                        nop["opcode"] = "NoOp"
                        nop["ins"] = []
                        nop["outs"] = []
                        nop["is_reset_sema"] = False
                        nop["sync_info"] = {
                            "on_update": [],
                            "on_wait": extra[ci : ci + max_waits],
                        }
                        new_insts.append(nop)
                new_insts.append(inst)
            blk["instructions"] = new_insts
    return m


def _patch_to_json(nc):
    orig = nc.to_json_bytes

    def patched(self):
        return json.dumps(_split_waits(json.loads(orig()))).encode()

    nc.to_json_bytes = types.MethodType(patched, nc)


def _bcast_ap(ap, parts):
    """[N]-shaped DRAM AP -> [parts, N] via zero-stride partition dim."""
    return bass.AP(
        tensor=ap.tensor, offset=ap.offset, ap=[[0, parts]] + list(ap.ap[-1:])
    )


# --------------------------------------------------------------------------
def build_nc():
    nc = bass.Bass()
    xD = nc.declare_dram_parameter("x", [S, E], F32, isOutput=False)
    # smA: bqk [P,8] ++ bv broadcast [P,512]; smB: gamma/beta/bo broadcasts
    smAD = nc.declare_dram_parameter("smA", [P, 8 + E], F32, isOutput=False)
    smBD = nc.declare_dram_parameter("smB", [P, 3 * E], F32, isOutput=False)
    # host-preprocessed layouts: x^T fp8, e-major prescaled fp8 weights,
    # Wo in 64-row head chunks (bf16, already /WS)
    xTD = nc.declare_dram_parameter("xT", [E, S], FP8, isOutput=False)
    wqpD = nc.declare_dram_parameter("Wq_p", [E, H * D], FP8, isOutput=False)
    wkpD = nc.declare_dram_parameter("Wk_p", [E, H * D], FP8, isOutput=False)
    wvpD = nc.declare_dram_parameter("Wv_p", [E, H * D], FP8, isOutput=False)
    wopD = nc.declare_dram_parameter("Wo_p", [D, H, E], FP8E5, isOutput=False)
    outD = nc.declare_dram_parameter("out", [S, E], F32, isOutput=True)

    qs = [nc.sync, nc.scalar, nc.gpsimd]  # DGE queues

    with tile.TileContext(nc) as tc:
        with (
            tc.tile_pool(name="persist", bufs=1) as persist,
        ):
            X = persist.tile([P, ST, E], F32, name="Xsb")
            XT = persist.tile([P, ET, S], FP8, name="XTsb")
            Wq_sb = persist.tile([P, ET, H, D], FP8, name="Wq_sb")
            Wk_sb = persist.tile([P, ET, H, D], FP8, name="Wk_sb")
            Wv_sb = persist.tile([P, ET, H * D], FP8, name="Wv_sb")
            Wo_sb = persist.tile([D, H, E], FP8E5, name="Wo_sb")
            smA = persist.tile([P, 8 + E], F32, name="smA")
            smB = persist.tile([P, 3 * E], F32, name="smB")
            bo_row = persist.tile([1, E], BF16, name="bo_row")
            ones_bf = persist.tile([1, E], BF16, name="ones_bf")
            ones8 = persist.tile([P, 2, D], FP8, name="ones8")
            bqk = smA[:, 0:8].rearrange("p (a b) -> p a b", a=2)
            bv_bc = smA[:, 8 : 8 + E].rearrange("p (h d) -> p h d", h=H)
            gamma_bc = smB[:, 0:E]
            beta_bc = smB[:, E : 2 * E]
            QT = persist.tile([P, NP, S], BF16, name="QTsb")
            KT = persist.tile([P, NP, S], BF16, name="KTsb")
            V8 = persist.tile([P, ST, H, D], FP8, name="V8sb")
            CCT = persist.tile([D, H, S], FP8, name="CCTsb")

            # ---------------- stage 0: loads + pair-0 Q/K -------------------
            with (
                tc.tile_pool(name="qkp", bufs=4, space="PSUM") as qkp,
            ):
                nc.vector.memset(ones8, 1.0)
                nc.vector.memset(ones_bf, 1.0)

                # PE warmup while the first DMAs land: HAM un-throttles after
                # ~3.4us of sustained activity; sized to end near the first
                # weight/x^T arrivals so the first projection isn't queued
                # behind idle warmups
                for _w in range(6):
                    warm = qkp.tile([P, 512], F32, tag="SC", name="warm")
                    nc.tensor.matmul(
                        warm, lhsT=ones_bf[:, 0:P], rhs=ones_bf[:, 0:512],
                        start=True, stop=True,
                    )

                # wave 1: one big DMA per queue slot (HWDGE charges ~0.6us
                # fixed per DMACopy; transfers fan out over 16 engines, so
                # few-and-large wins). sync: Wq + x^T half 1; scalar: Wk +
                # x^T half 2; gpsimd: biases + Wv.
                xt_re = lambda lo, hi: xTD[:, lo:hi].rearrange(
                    "(et p) s -> p et s", p=P
                )
                # pair-0 weight columns first: the shared DMA fabric drains
                # transfers in issue order, so the first-scores set must lead
                w_re = lambda wD, lo, hi: wD[:, lo:hi].rearrange(
                    "(et p) hd -> p et hd", p=P
                ).rearrange("p et (h d) -> p et h d", h=2 * (hi - lo) // P)
                nc.sync.dma_start(out=XT[:, :, 0:1024], in_=xt_re(0, 1024))
                nc.scalar.dma_start(
                    out=Wk_sb[:, :, 0:2], in_=w_re(wkpD, 0, P)
                )
                nc.gpsimd.dma_start(out=smA, in_=smAD[:])
                nc.gpsimd.dma_start(
                    out=Wq_sb[:, :, 0:2], in_=w_re(wqpD, 0, P)
                )
                nc.scalar.dma_start(
                    out=Wk_sb[:, :, 2:8], in_=w_re(wkpD, P, 4 * P)
                )
                nc.gpsimd.dma_start(
                    out=Wq_sb[:, :, 2:8], in_=w_re(wqpD, P, 4 * P)
                )
                nc.scalar.dma_start(
                    out=XT[:, :, 1024:2048], in_=xt_re(1024, 2048)
                )
                nc.sync.dma_start(
                    out=Wv_sb,
                    in_=wvpD[:].rearrange("(et p) hd -> p et hd", p=P),
                )

                # pair-0 projections, first-scores chunks only: Q cc0, K cc0,
                # Q cc1 (K cc1-3 stream in as block-0 chunks; Q cc2-3 later)
                for qk, cc in ((0, 0), (1, 0), (0, 1)):
                    wsb = Wq_sb if qk == 0 else Wk_sb
                    qt = QT if qk == 0 else KT
                    pq = qkp.tile([P, 512], F32, tag="SC", name="pq")
                    for ep in range(2):
                        nc.tensor.matmul(
                            pq,
                            lhsT=wsb[:, 2 * ep : 2 * ep + 2, 0:2]
                            .rearrange("p e h d -> p e (h d)"),
                            rhs=XT[
                                :, 2 * ep : 2 * ep + 2,
                                cc * 512 : (cc + 1) * 512,
                            ],
                            start=(ep == 0),
                            stop=(ep == 1),
                            perf_mode=PM.DoubleRow,
                        )
                    nc.vector.tensor_scalar_add(
                        qt[:, 0, cc * 512 : (cc + 1) * 512],
                        pq,
                        bqk[:, qk, 0:1],
                    )

            # ---------------- attention: 16 blocks of (sh, pp, hl) ----------
            with (
                tc.tile_pool(name="expp", bufs=5) as expp,
                tc.tile_pool(name="scp", bufs=2, space="PSUM") as scp,
                tc.tile_pool(name="ctxp", bufs=1, space="PSUM") as ctxp,
                tc.tile_pool(name="denp", bufs=1, space="PSUM") as denp,
                tc.tile_pool(name="recp", bufs=3) as recp,
                tc.tile_pool(name="outp", bufs=8) as outp,
                tc.tile_pool(name="statp", bufs=8) as statp,
            ):
                eps_t = statp.tile([P, 1], F32, tag="eps", bufs=1)
                nc.vector.memset(eps_t, LN_EPS)

                # ---- chunk emitters (scheduled through PE idle slots) ----
                def qk_chunk(pp, qk, cc):
                    def emit():
                        wsb = Wq_sb if qk == 0 else Wk_sb
                        qt = QT if qk == 0 else KT
                        pq = scp.tile([P, 1024], F32, tag="SC", name="pq2")
                        for ep in range(2):
                            nc.tensor.matmul(
                                pq[:, 0:512],
                                lhsT=wsb[:, 2 * ep : 2 * ep + 2,
                                         2 * pp : 2 * pp + 2]
                                .rearrange("p e h d -> p e (h d)"),
                                rhs=XT[
                                    :, 2 * ep : 2 * ep + 2,
                                    cc * 512 : (cc + 1) * 512,
                                ],
                                start=(ep == 0),
                                stop=(ep == 1),
                                perf_mode=PM.DoubleRow,
                            )
                        nc.vector.tensor_scalar_add(
                            qt[:, pp, cc * 512 : (cc + 1) * 512],
                            pq[:, 0:512],
                            bqk[:, qk, pp : pp + 1],
                        )

                    return emit

                def v_chunk(st):
                    def emit():
                        pv = scp.tile([P, 1024], F32, tag="SC", name="pv")
                        for ep in range(2):
                            nc.tensor.matmul(
                                pv[:, 0:512],
                                lhsT=XT[:, 2 * ep : 2 * ep + 2,
                                        st * P : (st + 1) * P],
                                rhs=Wv_sb[:, 2 * ep : 2 * ep + 2, :],
                                start=(ep == 0),
                                stop=(ep == 1),
                                perf_mode=PM.DoubleRow,
                            )
                        nc.vector.tensor_tensor(
                            V8[:, st],
                            pv[:, 0:512].rearrange("p (h d) -> p h d", h=H),
                            bv_bc,
                            OP.add,
                        )

                    return emit

                def st3_mm(st):
                    po = scp.tile([P, 1024], F32, tag="SC", name="po3")
                    for j in range(H // 2):
                        nc.tensor.matmul(
                            po[:, 0:E],
                            lhsT=CCT[:, 2 * j : 2 * j + 2,
                                     st * P : (st + 1) * P],
                            rhs=Wo_sb[:, 2 * j : 2 * j + 2, :],
                            start=(j == 0),
                            stop=False,
                            perf_mode=PM.DoubleRow,
                        )
                    nc.tensor.matmul(
                        po[:, 0:E], lhsT=ones_bf[:, 0:P], rhs=bo_row,
                        start=False, stop=True,
                    )
                    y = outp.tile([P, E], F32, tag="y", name="y")
                    nc.vector.tensor_add(y, po[:, 0:E], X[:, st])
                    return y

                def st3_ln_out(st, y, mu, rstd, tail=False):
                    nc.vector.tensor_scalar(
                        y, y, mu, rstd, OP.subtract, OP.mult
                    )
                    nc.vector.tensor_tensor(y, y, gamma_bc, OP.mult)
                    nc.gpsimd.tensor_tensor(y, y, beta_bc, OP.add)
                    nc.sync.dma_start(
                        out=outD[st * P : (st + 1) * P, :], in_=y
                    )

                def st3_chunk(st):
                    def emit():
                        y = st3_mm(st)
                        stats = statp.tile([P, 6], F32, tag="st", name="stats")
                        nc.vector.bn_stats(out=stats, in_=y)
                        mv = statp.tile([P, 2], F32, tag="mv", name="mv")
                        nc.vector.bn_aggr(out=mv, in_=stats)
                        rstd = statp.tile([P, 1], F32, tag="rs", name="rstd")
                        # rstd = exp(-0.5*ln(var+eps)): Ln/Exp share one ACT
                        # table set, so no table reloads between softmax exps
                        nc.scalar.activation(
                            out=rstd, in_=mv[:, 1:2], func=AF.Ln, bias=eps_t
                        )
                        nc.scalar.activation(
                            out=rstd, in_=rstd, func=AF.Exp, scale=-0.5
                        )
                        st3_ln_out(st, y, mv[:, 0:1], rstd)

                    return emit

                def st3_tail(st):
                    # epilogue variant: mean/var via ACT accumulators (ACT is
                    # idle after the last exp; DVE is the tail critical path)
                    y = st3_mm(st)
                    dump = statp.tile([P, E], BF16, tag="dmp", name="dump")
                    mu = statp.tile([P, 1], F32, tag="mu", name="mu")
                    nc.scalar.activation(
                        out=dump, in_=y, func=AF.Copy, scale=1.0 / E,
                        accum_out=mu,
                    )
                    dump2 = statp.tile([P, E], F32, tag="dm2", name="dump2")
                    a2 = statp.tile([P, 1], F32, tag="a2", name="a2")
                    nc.scalar.activation(
                        out=dump2, in_=y, func=AF.Square, scale=1.0 / E,
                        accum_out=a2,
                    )
                    musq = statp.tile([P, 1], F32, tag="mq", name="musq")
                    nc.vector.tensor_tensor(musq, mu, mu, OP.mult)
                    var = statp.tile([P, 1], F32, tag="va", name="var")
                    # var = E*sum((y/E)^2) - mu^2
                    nc.vector.tensor_scalar(
                        var, a2, float(E), musq, OP.mult, OP.subtract
                    )
                    rstd = statp.tile([P, 1], F32, tag="rs", name="rstd")
                    nc.scalar.activation(
                        out=rstd, in_=var, func=AF.Ln, bias=eps_t
                    )
                    nc.scalar.activation(
                        out=rstd, in_=rstd, func=AF.Exp, scale=-0.5
                    )
                    st3_ln_out(st, y, mu, rstd, tail=True)

                def wo_load():
                    def emit():
                        nc.gpsimd.dma_start(out=Wo_sb, in_=wopD[:])
                        nc.sync.dma_start(out=smB, in_=smBD[:])
                        nc.gpsimd.tensor_copy(
                            out=bo_row, in_=smB[0:1, 2 * E : 3 * E]
                        )

                    return emit

                def x_load(q):
                    def emit():
                        xDr = xD[:].rearrange("(st p) e -> p st e", p=P)
                        (nc.sync if q % 2 else nc.gpsimd).dma_start(
                            out=X[:, 4 * q : 4 * q + 4],
                            in_=xDr[:, 4 * q : 4 * q + 4],
                        )

                    return emit

                # ---- static chunk schedule ----
                pre = {}

                def add(b, t, fn):
                    pre.setdefault((b, t), []).append(fn)

                # pair-0 K tail: keys 512+ first consumed at t4/t8/t12
                add(0, 2, qk_chunk(0, 1, 1))
                add(0, 6, qk_chunk(0, 1, 2))
                add(0, 10, qk_chunk(0, 1, 3))
                for st in range(13):  # V projections: block 0
                    add(0, 3 + st, v_chunk(st))
                add(1, 1, v_chunk(13))
                add(1, 2, v_chunk(14))
                add(1, 3, v_chunk(15))
                qkseq = {  # pairs 1-3: K cc0-3 then Q cc0-1
                    1: [(0, 11), (0, 12), (0, 13), (0, 14), (1, 4), (1, 8)],
                    2: [(2, 4), (2, 8), (2, 12), (3, 4), (3, 8), (3, 12)],
                    3: [(4, 4), (4, 8), (4, 12), (5, 4), (5, 8), (5, 12)],
                }
                for pp, slots in qkseq.items():
                    needs = [(pp, 1, 0), (pp, 1, 1), (pp, 1, 2), (pp, 1, 3),
                             (pp, 0, 0), (pp, 0, 1)]
                    for (b, t), (p_, qk, cc) in zip(slots, needs):
                        add(b, t, qk_chunk(p_, qk, cc))
                qt23 = {0: [(5, 10), (6, 4)], 1: [(6, 8), (6, 12)],
                        2: [(8, 4), (8, 8)], 3: [(10, 4), (10, 8)]}
                for pp, slots in qt23.items():
                    for (b, t), cc in zip(slots, (2, 3)):
                        add(b, t, qk_chunk(pp, 0, cc))
                add(1, 6, wo_load())
                for q in range(4):
                    add(2 + q, 6, x_load(q))
                for st in range(8):  # stage-3 for the first s-half
                    add(8 + st, 8, st3_chunk(st))

                # ---- the 16 attention blocks ----
                blocks = [
                    (sh, pp, hl)
                    for sh in range(2)
                    for pp in range(NP)
                    for hl in range(2)
                ]
                prev = None  # (cx, den_t, h, s0) awaiting normalize

                def emit_ctx_den(cx, den_t, et2, k, h):
                    for cc in range(2):
                        nc.tensor.matmul(
                            cx[:, cc * 512 : (cc + 1) * 512],
                            lhsT=V8[:, 2 * k : 2 * k + 2, h, :],
                            rhs=et2[:, :, cc * 512 : (cc + 1) * 512],
                            start=(k == 0),
                            stop=(k == 7),
                            perf_mode=PM.DoubleRow,
                        )
                        nc.tensor.matmul(
                            den_t[:, cc * 512 : (cc + 1) * 512],
                            lhsT=ones8,
                            rhs=et2[:, :, cc * 512 : (cc + 1) * 512],
                            start=(k == 0),
                            stop=(k == 7),
                            perf_mode=PM.DoubleRow,
                        )

                def normalize_half(state, half):
                    cx_p, den_p, h_p, s0_p = state
                    lo, hi = half * 512, (half + 1) * 512
                    rec = recp.tile([D, 512], F32, tag="rec", name="rec")
                    nc.vector.reciprocal(rec, den_p[:, lo:hi])
                    nc.vector.tensor_tensor(
                        CCT[:, h_p, s0_p + lo : s0_p + hi],
                        cx_p[:, lo:hi],
                        rec,
                        OP.mult,
                    )

                def normalize(state):
                    cx_p, den_p, h_p, s0_p = state
                    rec = recp.tile([D, 1024], F32, tag="recf", name="recf")
                    nc.vector.reciprocal(rec, den_p)
                    nc.vector.tensor_tensor(
                        CCT[:, h_p, s0_p : s0_p + 1024], cx_p, rec, OP.mult
                    )

                for b, (sh, pp, hl) in enumerate(blocks):
                    s0 = sh * 1024
                    h = 2 * pp + hl
                    lo, hi = D * hl, D * (hl + 1)
                    cx = ctxp.tile([D, 1024], F32, tag="ctx", name="cx")
                    den_t = denp.tile([D, 1024], F32, tag="den", name="den")
                    ets = {}
                    for t in range(ST):
                        site = pre.get((b, t), [])
                        for fn in site:
                            fn()
                        if len(site) % 2:
                            # keep the SC ring parity: scores(t) must not
                            # land in scores(t-1)'s slot
                            scp.tile([P, 1024], F32, tag="SC", name="par")
                        if t == 1 and prev is not None:
                            normalize(prev)
                            prev = None
                        sc = scp.tile([P, 1024], F32, tag="SC", name="sc")
                        for cc in range(2):
                            nc.tensor.matmul(
                                sc[:, cc * 512 : (cc + 1) * 512],
                                lhsT=KT[lo:hi, pp, t * P : (t + 1) * P],
                                rhs=QT[
                                    lo:hi, pp,
                                    s0 + cc * 512 : s0 + (cc + 1) * 512,
                                ],
                                start=True,
                                stop=True,
                            )
                        if t % 2 == 0:
                            ets[t // 2] = expp.tile(
                                [P, 2, 1024], FP8, tag="expT", name="et"
                            )
                        nc.scalar.activation(
                            out=ets[t // 2][:, t % 2],
                            in_=sc,
                            func=AF.Exp,
                            scale=SCALE_EXP,
                        )
                        if t == 0 and b > 0:
                            # prev block's last ctx/den pair: emitted after
                            # this block's first scores so PE never stalls
                            # the exp pipeline on exp(t15) of the old block
                            pcx, pden, ph, ps0, pets = pending
                            emit_ctx_den(pcx, pden, pets, 7, ph)
                        if t >= 4 and t % 2 == 0:
                            k = (t - 4) // 2
                            emit_ctx_den(cx, den_t, ets.pop(k), k, h)
                    emit_ctx_den(cx, den_t, ets.pop(6), 6, h)
                    pending = (cx, den_t, h, s0, ets.pop(7))
                    prev = (cx, den_t, h, s0)

                # drain: last block's pair 7, its normalize, tail stage-3
                pcx, pden, ph, ps0, pets = pending
                emit_ctx_den(pcx, pden, pets, 7, ph)
                normalize_half(prev, 0)
                normalize_half(prev, 1)
                tc.cur_priority += 20000
                for st in range(8, ST):
                    st3_tail(st)
                tc.cur_priority -= 20000

    _patch_to_json(nc)
    return nc


_NC_CACHE = None


def _get_nc():
    global _NC_CACHE
    if _NC_CACHE is None:
        _NC_CACHE = build_nc()
    return _NC_CACHE


def kernel(**inputs) -> np.ndarray:
    import ml_dtypes
    from concourse.bass_utils import run_bass_kernel_spmd

    BF = ml_dtypes.bfloat16
    E4 = ml_dtypes.float8_e4m3fn
    nc = _get_nc()
    x = np.asarray(inputs["x"], dtype=np.float32)
    B = x.shape[0]

    def f32(k, scale=1.0):
        return np.ascontiguousarray(
            np.asarray(inputs[k], dtype=np.float32) * scale
        )

    def perm_w8(k):  # [H, E, D] -> [E, H*D] fp8, prescaled
        w = np.asarray(inputs[k], dtype=np.float32) * WS
        return np.ascontiguousarray(
            w.transpose(1, 0, 2).reshape(E, H * D).astype(E4)
        )

    bqk = np.stack(
        [
            np.asarray(inputs["bq"], np.float32).reshape(NP, P).T * WS,
            np.asarray(inputs["bk"], np.float32).reshape(NP, P).T * WS,
        ],
        axis=1,
    ).reshape(P, 8)
    bv_bc = np.broadcast_to(
        np.asarray(inputs["bv"], np.float32).reshape(1, H * D) * WS, (P, H * D)
    )
    smA = np.ascontiguousarray(np.concatenate([bqk, bv_bc], axis=1))
    smB = np.ascontiguousarray(
        np.stack(
            [
                np.broadcast_to(f32(k).reshape(1, E), (P, E))
                for k in ("gamma", "beta", "bo")
            ],
            axis=1,
        ).reshape(P, 3 * E)
    )
    wo = np.asarray(inputs["Wo"], np.float32) / WS  # [H*D, E]
    wo = wo.reshape(H, D, E).transpose(1, 0, 2)  # [D, H, E]
    shared = {
        "Wq_p": perm_w8("Wq"),
        "Wk_p": perm_w8("Wk"),
        "Wv_p": perm_w8("Wv"),
        "Wo_p": np.ascontiguousarray(wo.astype(ml_dtypes.float8_e5m2)),
        "smA": smA,
        "smB": smB,
    }
    in_maps = []
    for b in range(B):
        xb = np.ascontiguousarray(x[b])
        in_maps.append(
            {
                "x": xb,
                "xT": np.ascontiguousarray(xb.T.astype(E4)),
                **shared,
            }
        )
    res = run_bass_kernel_spmd(nc, in_maps, core_ids=list(range(B)))
    return np.stack([res.results[b]["out"] for b in range(B)], axis=0)
